# revision 1
# baseline (speedup 1.0000x reference)
"""2-layer LSTM (B=128, T=256, F=H=1024) on 8 Trainium2 NeuronCores.

Strategy: tensor-parallel over the 4H gate dimension. Each core owns a
512-wide gate slice (its 128-row slice of each of i, f, g, o) for the FULL
batch B=128 (batch = PSUM partition dim). Gates are computed as
    psum[B, 512] += xT_chunk[128, B].T @ W_chunk[128, 512]
(activations stationary, weights moving, N=512) in bf16 with fp32 PSUM
accumulation; the bias rides as a K=1 matmul. Each core updates its own
c/h slice locally in fp32, transposes h on the PE, and an 8-rank AllGather
exchanges bf16 h-chunks after each layer so every core has the full hidden
state for the next matmul. x is pre-transposed on the host; per-core
outputs (the core's 128-wide slice of h1 for every t) are concatenated on
the host.
"""

import numpy as np
import ml_dtypes

import concourse.bass as bass
import concourse.tile as tile
from concourse import mybir
from concourse.bass_utils import run_bass_kernel_spmd

N_CORES = 8
B = 128
T_FULL = 256
F = 1024
H = 1024
NG = 512          # gate columns per core (4 gates x 128)
KC_F = F // 128   # K chunks for x contraction
KC_H = H // 128   # K chunks for h contraction
BF16 = mybir.dt.bfloat16
F32 = mybir.dt.float32
AF = mybir.ActivationFunctionType


def _patch_tail_drain():
    """walrus on this image only allows ONE sem-wait on CTRL-type (Drain/NoOp)
    instructions; Tile's kernel-tail drain accumulates one wait per pending
    queue/collective sem and trips that limit. Spread the waits over a chain
    of single-wait nops instead."""
    if getattr(tile.TileContext, "_tail_drain_patched", False):
        return
    from concourse.tile import ScopedClock

    def _drain_and_barrier(self, tick_clock, wait_clock):
        nc = self.nc
        probe = nc.sync.nop(nofuse=True, hint="tail_wait_probe")
        wait_clock.add_sem_waits(probe.ins, ScopedClock({None: tick_clock.global_clock}))
        si = probe.ins.sync_info
        waits = list(si.on_wait) if si is not None else []
        if len(waits) > 1:
            si.on_wait = waits[:1]
            for w in waits[1:]:
                n2 = nc.sync.nop(nofuse=True, hint="tail_wait_extra")
                n2.ins.sync_info = mybir.SyncInfo(on_wait=[w], on_update=[])
        nc.sync.drain()
        nc.all_engine_barrier()
        popped = nc._tile_sem_poison_stack.pop()
        assert popped is self._sem_poison
        nc.clear_and_free_semaphores(list(self.sems.allocated().values()))
        nc.all_engine_barrier()

    tile.TileContext._drain_and_barrier = _drain_and_barrier
    tile.TileContext._tail_drain_patched = True


def _hoist_multi_waits(nc: bass.Bass):
    """walrus on this image rejects >1 sem-wait per instruction (CTRL and
    compute structs alike). Hoist extra waits onto single-wait NoOps inserted
    just before the instruction on the same engine (engine FIFO order makes
    this equivalent)."""
    for blk in nc.main_func.blocks:
        idx = 0
        while idx < len(blk.instructions):
            inst = blk.instructions[idx]
            si = getattr(inst, "sync_info", None)
            if si is not None and len(si.on_wait) > 1:
                waits = list(si.on_wait)
                si.on_wait = waits[-1:]
                for w in waits[:-1]:
                    nop = mybir.InstNoOp(
                        name=nc.get_next_instruction_name(), ins=[], outs=[]
                    )
                    nop.engine = inst.engine
                    nop.sync_info = mybir.SyncInfo(on_wait=[w], on_update=[])
                    nc.register_instruction(nop)
                    blk.instructions.insert(idx, nop)
                    idx += 1
            idx += 1


def build_lstm_nc(t_steps: int) -> bass.Bass:
    _patch_tail_drain()
    nc = bass.Bass()

    xT = nc.dram_tensor("xT", [t_steps, F, B], BF16, kind="ExternalInput")
    wx0 = nc.dram_tensor("wx0", [KC_F * 128, NG], BF16, kind="ExternalInput")
    wh0 = nc.dram_tensor("wh0", [KC_H * 128, NG], BF16, kind="ExternalInput")
    wx1 = nc.dram_tensor("wx1", [KC_H * 128, NG], BF16, kind="ExternalInput")
    wh1 = nc.dram_tensor("wh1", [KC_H * 128, NG], BF16, kind="ExternalInput")
    bias0 = nc.dram_tensor("bias0", [1, NG], BF16, kind="ExternalInput")
    bias1 = nc.dram_tensor("bias1", [1, NG], BF16, kind="ExternalInput")
    ones = nc.dram_tensor("ones", [1, 128], BF16, kind="ExternalInput")
    ident = nc.dram_tensor("ident", [128, 128], F32, kind="ExternalInput")
    y = nc.dram_tensor("y", [B, t_steps, 128], F32, kind="ExternalOutput")

    rg = [list(range(N_CORES))]

    with tile.TileContext(nc) as tc:
        with (
            tc.tile_pool(name="const", bufs=1) as cpool,
            tc.tile_pool(name="xbuf", bufs=3) as xpool,
            tc.tile_pool(name="hh", bufs=2) as hhpool,
            tc.tile_pool(name="state", bufs=1) as spool,
            tc.tile_pool(name="act", bufs=2) as apool,
            tc.tile_pool(name="pg0", bufs=2, space="PSUM") as pg0,
            tc.tile_pool(name="pg1", bufs=2, space="PSUM") as pg1,
            tc.tile_pool(name="ptr", bufs=2, space="PSUM") as ptr,
            tc.tile_pool(name="dram", bufs=2, space="DRAM") as dpool,
        ):
            # ---- constants resident in SBUF
            wx0_sb = cpool.tile([128, KC_F * NG], BF16)
            wh0_sb = cpool.tile([128, KC_H * NG], BF16)
            wx1_sb = cpool.tile([128, KC_H * NG], BF16)
            wh1_sb = cpool.tile([128, KC_H * NG], BF16)
            for c in range(KC_F):
                nc.sync.dma_start(wx0_sb[:, c * NG:(c + 1) * NG], wx0[c * 128:(c + 1) * 128, :])
            for c in range(KC_H):
                nc.sync.dma_start(wh0_sb[:, c * NG:(c + 1) * NG], wh0[c * 128:(c + 1) * 128, :])
                nc.sync.dma_start(wx1_sb[:, c * NG:(c + 1) * NG], wx1[c * 128:(c + 1) * 128, :])
                nc.sync.dma_start(wh1_sb[:, c * NG:(c + 1) * NG], wh1[c * 128:(c + 1) * 128, :])
            b0_sb = cpool.tile([1, NG], BF16)
            b1_sb = cpool.tile([1, NG], BF16)
            ones_sb = cpool.tile([1, 128], BF16)
            id_sb = cpool.tile([128, 128], F32)
            nc.sync.dma_start(b0_sb[:], bias0[:])
            nc.sync.dma_start(b1_sb[:], bias1[:])
            nc.sync.dma_start(ones_sb[:], ones[:])
            nc.sync.dma_start(id_sb[:], ident[:])

            # ---- persistent state (fp32 cell state; zero-init)
            c0_sb = spool.tile([128, 128], F32)
            c1_sb = spool.tile([128, 128], F32)
            nc.vector.memset(c0_sb[:], 0.0)
            nc.vector.memset(c1_sb[:], 0.0)
            # gathered hidden states (bf16, [128, KC*128]); zero for t=0
            hh0_prev = hhpool.tile([128, KC_H * 128], BF16, tag="hh0")
            hh1_prev = hhpool.tile([128, KC_H * 128], BF16, tag="hh1")
            nc.vector.memset(hh0_prev[:], 0.0)
            nc.vector.memset(hh1_prev[:], 0.0)

            def cell(ps, cst, if_w=256):
                """LSTM pointwise math on psum gates [128, 512] (i|f|g|o).
                Returns fp32 h tile."""
                ifp = apool.tile([128, 256], F32, tag="ifp")
                gp = apool.tile([128, 128], F32, tag="gp")
                op = apool.tile([128, 128], F32, tag="op")
                nc.scalar.activation(ifp[:], ps[:, 0:256], AF.Sigmoid)
                nc.scalar.activation(gp[:], ps[:, 256:384], AF.Tanh)
                nc.scalar.activation(op[:], ps[:, 384:512], AF.Sigmoid)
                t1 = apool.tile([128, 128], F32, tag="t1")
                nc.vector.tensor_mul(t1[:], ifp[:, 0:128], gp[:])      # i*g
                nc.vector.tensor_mul(cst[:], cst[:], ifp[:, 128:256])  # c *= f
                nc.vector.tensor_add(cst[:], cst[:], t1[:])            # c += i*g
                th = apool.tile([128, 128], F32, tag="th")
                nc.scalar.activation(th[:], cst[:], AF.Tanh)
                hf = apool.tile([128, 128], F32, tag="hf")
                nc.vector.tensor_mul(hf[:], op[:], th[:])              # h = o*tanh(c)
                return hf

            def exchange(hf, tagbase):
                """transpose h -> bf16 hT chunk, AllGather, load gathered [128, KC*128]."""
                tr = ptr.tile([128, 128], F32, tag="tr")
                nc.tensor.transpose(tr[:], hf[:], id_sb[:])
                hT = apool.tile([128, 128], BF16, tag=f"{tagbase}T")
                nc.scalar.activation(hT[:], tr[:], AF.Copy)
                cc_in = dpool.tile([128, 128], BF16, tag=f"{tagbase}ci")
                cc_out = dpool.tile([N_CORES * 128, 128], BF16, addr_space="Shared",
                                    tag=f"{tagbase}co")
                nc.sync.dma_start(cc_in[:], hT[:])
                nc.gpsimd.collective_compute(
                    "AllGather", mybir.AluOpType.bypass,
                    ins=[cc_in.opt()], outs=[cc_out.opt()], replica_groups=rg,
                )
                hh = hhpool.tile([128, KC_H * 128], BF16, tag=tagbase)
                for c in range(KC_H):
                    nc.sync.dma_start(hh[:, c * 128:(c + 1) * 128],
                                      cc_out[c * 128:(c + 1) * 128, :])
                return hh

            for t in range(t_steps):
                # x slab for step t: [128, KC_F*128] from xT[t] ([F, B] in DRAM)
                x_sb = xpool.tile([128, KC_F * 128], BF16, tag="x")
                for c in range(KC_F):
                    nc.sync.dma_start(x_sb[:, c * 128:(c + 1) * 128],
                                      xT[t, c * 128:(c + 1) * 128, :])

                # ---- layer 0 gates
                g0 = pg0.tile([128, NG], F32, tag="g0")
                nc.tensor.matmul(g0[:], ones_sb[:], b0_sb[:], start=True, stop=False)
                for c in range(KC_F):
                    nc.tensor.matmul(g0[:], x_sb[:, c * 128:(c + 1) * 128],
                                     wx0_sb[:, c * NG:(c + 1) * NG],
                                     start=False, stop=False)
                for c in range(KC_H):
                    nc.tensor.matmul(g0[:], hh0_prev[:, c * 128:(c + 1) * 128],
                                     wh0_sb[:, c * NG:(c + 1) * NG],
                                     start=False, stop=(c == KC_H - 1))
                h0f = cell(g0, c0_sb)
                hh0_prev = exchange(h0f, "hh0")

                # ---- layer 1 gates (input = h0(t) via gathered hh0_prev)
                g1 = pg1.tile([128, NG], F32, tag="g1")
                nc.tensor.matmul(g1[:], ones_sb[:], b1_sb[:], start=True, stop=False)
                for c in range(KC_H):
                    nc.tensor.matmul(g1[:], hh1_prev[:, c * 128:(c + 1) * 128],
                                     wh1_sb[:, c * NG:(c + 1) * NG],
                                     start=False, stop=False)
                for c in range(KC_H):
                    nc.tensor.matmul(g1[:], hh0_prev[:, c * 128:(c + 1) * 128],
                                     wx1_sb[:, c * NG:(c + 1) * NG],
                                     start=False, stop=(c == KC_H - 1))
                h1f = cell(g1, c1_sb)
                nc.sync.dma_start(y[:, t, :], h1f[:])
                hh1_prev = exchange(h1f, "hh1")

    _hoist_multi_waits(nc)
    return nc


def _prep_inputs(x, W_ih0, b_ih0, W_hh0, b_hh0, W_ih1, b_ih1, W_hh1, b_hh1,
                 t_steps: int):
    """Host-side: transpose x, slice/convert weights per core."""
    bf = ml_dtypes.bfloat16
    xT = np.ascontiguousarray(np.transpose(x[:, :t_steps, :], (1, 2, 0))).astype(bf)
    ones = np.ones((1, 128), np.float32).astype(bf)
    ident = np.eye(128, dtype=np.float32)

    in_maps = []
    for k in range(N_CORES):
        rows = np.concatenate([np.arange(g * H + k * 128, g * H + (k + 1) * 128)
                               for g in range(4)])
        # moving weight layout: [K, 512] = W[rows, :].T
        wx0 = np.ascontiguousarray(W_ih0[rows, :].T).astype(bf)
        wh0 = np.ascontiguousarray(W_hh0[rows, :].T).astype(bf)
        wx1 = np.ascontiguousarray(W_ih1[rows, :].T).astype(bf)
        wh1 = np.ascontiguousarray(W_hh1[rows, :].T).astype(bf)
        b0 = (b_ih0 + b_hh0)[rows][None, :].astype(bf)
        b1 = (b_ih1 + b_hh1)[rows][None, :].astype(bf)
        in_maps.append({
            "xT": xT, "wx0": wx0, "wh0": wh0, "wx1": wx1, "wh1": wh1,
            "bias0": b0, "bias1": b1, "ones": ones, "ident": ident,
        })
    return in_maps


def run_lstm(inputs: dict, t_steps: int = T_FULL, trace: bool = False):
    """Build, run on 8 cores, return (y_full, BassKernelResults)."""
    in_maps = _prep_inputs(**inputs, t_steps=t_steps)
    nc = build_lstm_nc(t_steps)
    res = run_bass_kernel_spmd(nc, in_maps, list(range(N_CORES)), trace=trace)
    y = np.concatenate([res.results[k]["y"] for k in range(N_CORES)], axis=2)
    return y, res


def kernel(**inputs) -> np.ndarray:
    y, _ = run_lstm(inputs, t_steps=T_FULL, trace=False)
    return y


if __name__ == "__main__":
    # quick self-run with random inputs at reduced T
    rng = np.random.default_rng(0)
    sc = 1.0 / np.sqrt(F)
    ins = {
        "x": rng.standard_normal((B, T_FULL, F)).astype(np.float32),
        "W_ih0": (rng.standard_normal((4 * H, F)) * sc).astype(np.float32),
        "b_ih0": (rng.standard_normal(4 * H) * sc).astype(np.float32),
        "W_hh0": (rng.standard_normal((4 * H, H)) * sc).astype(np.float32),
        "b_hh0": (rng.standard_normal(4 * H) * sc).astype(np.float32),
        "W_ih1": (rng.standard_normal((4 * H, H)) * sc).astype(np.float32),
        "b_ih1": (rng.standard_normal(4 * H) * sc).astype(np.float32),
        "W_hh1": (rng.standard_normal((4 * H, H)) * sc).astype(np.float32),
        "b_hh1": (rng.standard_normal(4 * H) * sc).astype(np.float32),
    }
    y, res = run_lstm(ins, t_steps=8)
    print("y shape", y.shape, "exec_time_ns", res.exec_time_ns)



# revision 2
# speedup vs baseline: 9008.5181x; 9008.5181x over previous
"""2-layer LSTM (B=128, T=256, F=H=1024) on 8 Trainium2 NeuronCores.

Tensor-parallel over the 4H gate dimension (each core owns a 512-wide
[i|f|o|g]-packed gate slice for the full batch; batch is the PSUM partition
dim). Two AllGathers per step (h0, h1), software-pipelined so that the
consumers of AG0(t) immediately produce the payloads of AG1(t) and
AG0(t+1):

  iteration t:  recv hh0(t) -> [wx1(c) | wh0(c)] chunk-paced matmuls
                -> cell1 -> send AG1(t)   (h1(t))
                -> cell0 -> send AG0(t+1) (h0(t+1))

- x-part of layer-0 gates is accumulated into PSUM PRE steps ahead (bias +
  x @ W_ih0) so the PE has independent work during collective waits.
- x slabs are host-relayered to [t, p, c, b] so each step's slab is ONE
  contiguous 256KB DMA.
- gathered-h loads are split 4+4 across the sync/scalar queues; the
  consuming matmuls are chunk-paced (each waits only for its own chunk).
- queues: sync = hh0[0:4]/cc_in1/hh1[0:4]; scalar = ACT + hh0[4:8]/y/
  cc_in0/hh1[4:8]/x-slab; vector = cell arithmetic + bf16 casts;
  gpsimd = collective triggers only.
"""

import numpy as np
import ml_dtypes

import concourse.bass as bass
import concourse.tile as tile
from concourse import mybir
from concourse.bass_utils import run_bass_kernel_spmd

N_CORES = 8
B = 128
T_FULL = 256
F = 1024
H = 1024
NG = 512          # gate columns per core, packed [i|f|o|g]
KC_F = F // 128
KC_H = H // 128
PRE = 3           # x-part prefetch depth (steps ahead)
BF16 = mybir.dt.bfloat16
F32 = mybir.dt.float32
AF = mybir.ActivationFunctionType


def _patch_tail_drain():
    """walrus on this image only allows ONE sem-wait on CTRL-type (Drain/NoOp)
    instructions; Tile's kernel-tail drain accumulates one wait per pending
    queue/collective sem and trips that limit. Spread the waits over a chain
    of single-wait nops instead."""
    if getattr(tile.TileContext, "_tail_drain_patched", False):
        return
    from concourse.tile import ScopedClock

    def _drain_and_barrier(self, tick_clock, wait_clock):
        nc = self.nc
        probe = nc.sync.nop(nofuse=True, hint="tail_wait_probe")
        wait_clock.add_sem_waits(probe.ins, ScopedClock({None: tick_clock.global_clock}))
        si = probe.ins.sync_info
        waits = list(si.on_wait) if si is not None else []
        if len(waits) > 1:
            si.on_wait = waits[:1]
            for w in waits[1:]:
                n2 = nc.sync.nop(nofuse=True, hint="tail_wait_extra")
                n2.ins.sync_info = mybir.SyncInfo(on_wait=[w], on_update=[])
        nc.sync.drain()
        nc.all_engine_barrier()
        popped = nc._tile_sem_poison_stack.pop()
        assert popped is self._sem_poison
        nc.clear_and_free_semaphores(list(self.sems.allocated().values()))
        nc.all_engine_barrier()

    tile.TileContext._drain_and_barrier = _drain_and_barrier
    tile.TileContext._tail_drain_patched = True


def _hoist_multi_waits(nc: bass.Bass):
    """walrus on this image rejects >1 sem-wait per instruction (CTRL and
    compute structs alike). Hoist extra waits onto single-wait NoOps inserted
    just before the instruction on the same engine (engine FIFO order makes
    this equivalent)."""
    for blk in nc.main_func.blocks:
        idx = 0
        while idx < len(blk.instructions):
            inst = blk.instructions[idx]
            si = getattr(inst, "sync_info", None)
            if si is not None and len(si.on_wait) > 1:
                waits = list(si.on_wait)
                si.on_wait = waits[-1:]
                for w in waits[:-1]:
                    nop = mybir.InstNoOp(
                        name=nc.get_next_instruction_name(), ins=[], outs=[]
                    )
                    nop.engine = inst.engine
                    nop.sync_info = mybir.SyncInfo(on_wait=[w], on_update=[])
                    nc.register_instruction(nop)
                    blk.instructions.insert(idx, nop)
                    idx += 1
            idx += 1


def build_lstm_nc(t_steps: int) -> bass.Bass:
    _patch_tail_drain()
    nc = bass.Bass()

    # x relayered on host: xT2[t, p, c, b] = x[b, t, c*128+p]
    xT2 = nc.dram_tensor("xT2", [t_steps, 128, KC_F, B], BF16, kind="ExternalInput")
    wx0 = nc.dram_tensor("wx0", [KC_F * 128, NG], BF16, kind="ExternalInput")
    wh0 = nc.dram_tensor("wh0", [KC_H * 128, NG], BF16, kind="ExternalInput")
    wx1 = nc.dram_tensor("wx1", [KC_H * 128, NG], BF16, kind="ExternalInput")
    wh1 = nc.dram_tensor("wh1", [KC_H * 128, NG], BF16, kind="ExternalInput")
    bias0 = nc.dram_tensor("bias0", [1, NG], BF16, kind="ExternalInput")
    bias1 = nc.dram_tensor("bias1", [1, NG], BF16, kind="ExternalInput")
    ones = nc.dram_tensor("ones", [1, 128], BF16, kind="ExternalInput")
    ident = nc.dram_tensor("ident", [128, 128], F32, kind="ExternalInput")
    y = nc.dram_tensor("y", [B, t_steps, 128], F32, kind="ExternalOutput")

    rg = [list(range(N_CORES))]

    with tile.TileContext(nc) as tc:
        with (
            tc.tile_pool(name="const", bufs=1) as cpool,
            tc.tile_pool(name="xbuf", bufs=3) as xpool,
            tc.tile_pool(name="hh", bufs=2) as hhpool,
            tc.tile_pool(name="state", bufs=1) as spool,
            tc.tile_pool(name="act", bufs=3) as apool,
            tc.tile_pool(name="pg0", bufs=PRE + 1, space="PSUM") as pg0,
            tc.tile_pool(name="pg1", bufs=2, space="PSUM") as pg1,
            tc.tile_pool(name="ptr", bufs=2, space="PSUM") as ptr,
            tc.tile_pool(name="dram", bufs=2, space="DRAM") as dpool,
        ):
            # ---- constants resident in SBUF
            wx0_sb = cpool.tile([128, KC_F * NG], BF16)
            wh0_sb = cpool.tile([128, KC_H * NG], BF16)
            wx1_sb = cpool.tile([128, KC_H * NG], BF16)
            wh1_sb = cpool.tile([128, KC_H * NG], BF16)
            for c in range(KC_F):
                nc.sync.dma_start(wx0_sb[:, c * NG:(c + 1) * NG], wx0[c * 128:(c + 1) * 128, :])
            for c in range(KC_H):
                nc.sync.dma_start(wh0_sb[:, c * NG:(c + 1) * NG], wh0[c * 128:(c + 1) * 128, :])
                nc.sync.dma_start(wx1_sb[:, c * NG:(c + 1) * NG], wx1[c * 128:(c + 1) * 128, :])
                nc.sync.dma_start(wh1_sb[:, c * NG:(c + 1) * NG], wh1[c * 128:(c + 1) * 128, :])
            b0_sb = cpool.tile([1, NG], BF16)
            b1_sb = cpool.tile([1, NG], BF16)
            ones_sb = cpool.tile([1, 128], BF16)
            id_sb = cpool.tile([128, 128], F32)
            nc.sync.dma_start(b0_sb[:], bias0[:])
            nc.sync.dma_start(b1_sb[:], bias1[:])
            nc.sync.dma_start(ones_sb[:], ones[:])
            nc.sync.dma_start(id_sb[:], ident[:])

            # ---- persistent state (fp32 cell state; zero-init)
            c0_sb = spool.tile([128, 128], F32)
            c1_sb = spool.tile([128, 128], F32)
            nc.vector.memset(c0_sb[:], 0.0)
            nc.vector.memset(c1_sb[:], 0.0)

            def load_x_and_start_g0(t, close_group=False):
                """One contiguous SWDGE DMA (idle Q7; off the HWDGE rings) for
                the step-t x slab, then bias + x-part into a fresh pg0
                accumulation group (recurrence-independent)."""
                x_sb = xpool.tile([128, KC_F * 128], BF16, tag="x")
                nc.gpsimd.dma_start(
                    x_sb[:].rearrange("p (c b) -> p c b", c=KC_F, b=B),
                    xT2[t],
                )
                g0 = pg0.tile([128, NG], F32, tag="g0")
                nc.tensor.matmul(g0[:], ones_sb[:], b0_sb[:], start=True, stop=False)
                for c in range(KC_F):
                    nc.tensor.matmul(g0[:], x_sb[:, c * 128:(c + 1) * 128],
                                     wx0_sb[:, c * NG:(c + 1) * NG],
                                     start=False,
                                     stop=close_group and c == KC_F - 1)
                return g0

            def cell(ps, cst):
                """pointwise math on psum gates [128, 512] packed [i|f|o|g]."""
                ifo = apool.tile([128, 384], F32, tag="ifo")
                gp = apool.tile([128, 128], F32, tag="gp")
                nc.scalar.activation(ifo[:], ps[:, 0:384], AF.Sigmoid)
                nc.scalar.activation(gp[:], ps[:, 384:512], AF.Tanh)
                t1 = apool.tile([128, 128], F32, tag="t1")
                nc.vector.tensor_mul(t1[:], ifo[:, 0:128], gp[:])      # i*g
                nc.vector.tensor_mul(cst[:], cst[:], ifo[:, 128:256])  # c *= f
                nc.vector.tensor_add(cst[:], cst[:], t1[:])            # c += i*g
                th = apool.tile([128, 128], F32, tag="th")
                nc.scalar.activation(th[:], cst[:], AF.Tanh)
                hf = apool.tile([128, 128], F32, tag="hf")
                nc.vector.tensor_mul(hf[:], ifo[:, 256:384], th[:])    # h = o*tanh(c)
                return hf

            def send(hf, tagbase, dma_eng):
                """transpose h -> bf16 hT -> DRAM -> AllGather (async)."""
                tr = ptr.tile([128, 128], F32, tag="tr")
                nc.tensor.transpose(tr[:], hf[:], id_sb[:])
                hT = apool.tile([128, 128], BF16, tag=f"{tagbase}T")
                nc.vector.tensor_copy(hT[:], tr[:])
                cc_in = dpool.tile([128, 128], BF16, tag=f"{tagbase}ci")
                cc_out = dpool.tile([N_CORES * 128, 128], BF16, addr_space="Shared",
                                    tag=f"{tagbase}co")
                dma_eng.dma_start(cc_in[:], hT[:])
                nc.gpsimd.collective_compute(
                    "AllGather", mybir.AluOpType.bypass,
                    ins=[cc_in.opt()], outs=[cc_out.opt()], replica_groups=rg,
                )
                return cc_out

            def recv(cc_out, tagbase):
                """gathered [128, KC*128] in four 2-chunk strided DMAs,
                alternating sync/scalar rings (finer pacing for the
                chunk-consuming matmuls keeps the PE warm)."""
                hh = hhpool.tile([128, KC_H * 128], BF16, tag=tagbase)
                grp = 2
                for i in range(KC_H // grp):
                    eng = nc.sync if i % 2 == 0 else nc.scalar
                    dst = hh[:, i * grp * 128:(i + 1) * grp * 128]
                    src = cc_out[i * grp * 128:(i + 1) * grp * 128, :]
                    eng.dma_start(
                        dst.rearrange("p (c b) -> p c b", c=grp, b=128),
                        src.rearrange("(c p) b -> p c b", c=grp, p=128),
                    )
                return hh

            # ---- prologue: x-parts for steps 0..PRE-1; h0(0) has no h-part
            # (h(-1)=0), so step 0's group closes at its last x chunk.
            g0_tiles = {}
            for s in range(min(PRE, t_steps)):
                g0_tiles[s] = load_x_and_start_g0(s, close_group=(s == 0))
            g0 = g0_tiles.pop(0)
            h0f = cell(g0, c0_sb)
            cc0 = send(h0f, "hh0", nc.scalar)

            hh1_prev = None
            for t in range(t_steps):
                last = t == t_steps - 1
                hh0 = recv(cc0, "hh0")

                # layer-1 gates: bias + wh1 (h1(t-1)) + wx1 (h0(t), chunk-paced)
                g1 = pg1.tile([128, NG], F32, tag="g1")
                nc.tensor.matmul(g1[:], ones_sb[:], b1_sb[:], start=True, stop=False)
                if hh1_prev is not None:
                    for c in range(KC_H):
                        nc.tensor.matmul(g1[:], hh1_prev[:, c * 128:(c + 1) * 128],
                                         wh1_sb[:, c * NG:(c + 1) * NG],
                                         start=False, stop=False)
                # wh0 first: h0(t+1) is the critical chain (its AllGather
                # gates the next step), so its flight starts ~5us before
                # AG1(t)'s and the two serialize without stretching the period.
                g0 = g0_tiles.pop(t + 1, None)
                if g0 is not None:
                    for c in range(KC_H):
                        nc.tensor.matmul(g0[:], hh0[:, c * 128:(c + 1) * 128],
                                         wh0_sb[:, c * NG:(c + 1) * NG],
                                         start=False, stop=(c == KC_H - 1))
                    h0f = cell(g0, c0_sb)
                    cc0 = send(h0f, "hh0", nc.scalar)
                for c in range(KC_H):
                    nc.tensor.matmul(g1[:], hh0[:, c * 128:(c + 1) * 128],
                                     wx1_sb[:, c * NG:(c + 1) * NG],
                                     start=False, stop=(c == KC_H - 1))
                h1f = cell(g1, c1_sb)
                nc.gpsimd.dma_start(y[:, t, :], h1f[:])
                if not last:
                    cc1 = send(h1f, "hh1", nc.sync)
                    if t + PRE < t_steps:
                        g0_tiles[t + PRE] = load_x_and_start_g0(t + PRE)
                    hh1_prev = recv(cc1, "hh1")

    _hoist_multi_waits(nc)
    return nc


def _prep_inputs(x, W_ih0, b_ih0, W_hh0, b_hh0, W_ih1, b_ih1, W_hh1, b_hh1,
                 t_steps: int):
    """Host-side: relayer x, slice/convert weights per core. Gate rows packed
    [i|f|o|g] so the kernel can sigmoid 384 contiguous columns."""
    bf = ml_dtypes.bfloat16
    # xT2[t, p, c, b] = x[b, t, c*128+p]
    xs = x[:, :t_steps, :].reshape(B, t_steps, KC_F, 128)
    xT2 = np.ascontiguousarray(np.transpose(xs, (1, 3, 2, 0))).astype(bf)
    ones = np.ones((1, 128), np.float32).astype(bf)
    ident = np.eye(128, dtype=np.float32)

    in_maps = []
    for k in range(N_CORES):
        rows = np.concatenate([np.arange(g * H + k * 128, g * H + (k + 1) * 128)
                               for g in (0, 1, 3, 2)])  # i, f, o, g
        wx0 = np.ascontiguousarray(W_ih0[rows, :].T).astype(bf)
        wh0 = np.ascontiguousarray(W_hh0[rows, :].T).astype(bf)
        wx1 = np.ascontiguousarray(W_ih1[rows, :].T).astype(bf)
        wh1 = np.ascontiguousarray(W_hh1[rows, :].T).astype(bf)
        b0 = (b_ih0 + b_hh0)[rows][None, :].astype(bf)
        b1 = (b_ih1 + b_hh1)[rows][None, :].astype(bf)
        in_maps.append({
            "xT2": xT2, "wx0": wx0, "wh0": wh0, "wx1": wx1, "wh1": wh1,
            "bias0": b0, "bias1": b1, "ones": ones, "ident": ident,
        })
    return in_maps


def run_lstm(inputs: dict, t_steps: int = T_FULL, trace: bool = False):
    in_maps = _prep_inputs(**inputs, t_steps=t_steps)
    nc = build_lstm_nc(t_steps)
    res = run_bass_kernel_spmd(nc, in_maps, list(range(N_CORES)), trace=trace)
    y = np.concatenate([res.results[k]["y"] for k in range(N_CORES)], axis=2)
    return y, res


def kernel(**inputs) -> np.ndarray:
    y, _ = run_lstm(inputs, t_steps=T_FULL, trace=False)
    return y
